# revision 45
# baseline (speedup 1.0000x reference)
"""Trainium2 Bass kernel for a binarized ResNet BasicBlock (stride-2).

Reference computation (per image):
    residual = BN2(conv1x1(avgpool2x2(x), w_ds))          # full precision
    body     = BN1(conv3x3_s2_p1(sign(x), sign(w_body)))  # binarized
    out      = body + residual

Shapes: x [16, 32, 224, 224] f32 -> out [16, 64, 112, 112] f32.
Sharding: data-parallel over batch, 2 images per core on 8 cores.

Per-core kernel layout (per 16-output-row chunk):
  * One cast-DMA (f32->bf16) loads input rows into V: partition par*32+ci
    holds row 2*Yq+par of channel ci.
  * S holds sign(x) as +-1 bf16: one fused DVE tensor_scalar computes
    (v & 0x8000) | 0x3f80 on uint16 views. Zero-pad columns u' in {0,1} of
    S are initialized once per physical buffer and never rewritten; tap kx
    reads u' = 2X+kx+1, so kx=0 at X=0 reads zero padding.
  * Per 4-output-row tile, matmuls accumulate into one PSUM bank:
    3 kx taps of (ky1, ky2) as K=64 over the chunk's sign partitions,
    3 kx taps of ky=0 as K=32 reading the odd-row (par=1) sign quarter one
    row slot back (no data duplication), and 2 residual matmuls (one per
    dx, rhs = V, weights pre-scaled by inv2/(4*inv1)); then one ScalarE
    activation (Identity, per-partition scale/bias vectors) applies both
    BNs while evacuating PSUM->SBUF f32, and one DMA stores the chunk.
  * Chunks alternate between the two partition halves / PE column groups
    so DMAs spread across both SDMA engine halves and consecutive chunks'
    matmuls can overlap in the PE array (column-group tiling).
"""

import numpy as np
import ml_dtypes

EPS = 1e-5

# Full-problem constants (hardcoded; the harness provides only kernel.py).
B, CIN, COUT, H, W = 16, 32, 64, 224, 224
N_CORES = 8
B_CORE = B // N_CORES  # 2 images per core


def build_nc(b_core=B_CORE, cin=CIN, cout=COUT, h=H, w=W, chunk_rows=16,
             loop_reps=1, ablate=None, in_path="swdge"):
    """Build the Bass program for one core processing b_core images.

    loop_reps > 1 wraps the whole computation in a hardware loop (identical
    results each iteration) — used only for wall-clock timing amplification.
    """
    from contextlib import nullcontext
    import concourse.bass as bass
    import concourse.bacc as bacc
    import concourse.mybir as mybir
    import concourse.tile as tile

    ho, wo = h // 2, w // 2
    assert ho % chunk_rows == 0
    n_chunks = ho // chunk_rows
    assert chunk_rows % 4 == 0
    T = chunk_rows // 4  # 4 output rows per matmul tile
    nslots = chunk_rows + 1  # one extra leading row slot per chunk

    f32 = mybir.dt.float32
    bf16 = mybir.dt.bfloat16
    u16 = mybir.dt.uint16

    nc = bacc.Bacc("TRN2", target_bir_lowering=False, debug=False)

    # Input is pre-arranged on the host as one payload per chunk PAIR:
    # zz[pair, p, slot, u] where partitions 0:64 hold the even chunk's rows
    # ((par, ci) major, slot = leading-row + 16 rows) and 64:128 the odd
    # chunk's, so a single fully-contiguous 128-partition cast-DMA feeds two
    # chunks (all 16 SDMA engines engaged).
    hh = h // 2
    n_pairs = (b_core * n_chunks + 1) // 2
    zz = nc.dram_tensor(
        "zz", [n_pairs, 128, nslots, w], f32, kind="ExternalInput"
    )
    # Body weights: w_body_t = (ky1, ky2) rows, w_body_t2 = ky0 rows.
    w_body_t = nc.dram_tensor("w_body_t", [2 * cin, 3, cout], bf16, kind="ExternalInput")
    w_body_t2 = nc.dram_tensor("w_body_t2", [cin, 3, cout], bf16, kind="ExternalInput")
    w_res_t = nc.dram_tensor("w_res_t", [2 * cin, cout], bf16, kind="ExternalInput")
    bn_scale = nc.dram_tensor("bn_scale", [cout, 1], f32, kind="ExternalInput")
    bn_bias = nc.dram_tensor("bn_bias", [cout, 1], f32, kind="ExternalInput")
    out = nc.dram_tensor("out", [b_core, cout, ho, wo], f32, kind="ExternalOutput")



    with tile.TileContext(nc) as tc:
        with tc.tile_pool(name="consts", bufs=1) as cpool:
            # Body weights: the direct taps (ky1, ky2) feed K=64 matmuls over
            # the parity's own partition half; the ky=0 tap reads the odd-row
            # sign partitions directly (one row-slot back) as K=32 matmuls,
            # so its weights sit on the par=1 sub-range of each half.
            wba = cpool.tile([2 * cin, 3, cout], bf16)
            nc.sync.dma_start(out=wba[:, :, :], in_=w_body_t.ap()[:, :, :])
            wbb = cpool.tile([4 * cin, 3, cout], bf16)
            nc.sync.dma_start(out=wbb[2 * cin : 4 * cin, :, :], in_=w_body_t.ap()[:, :, :])
            wk0a = cpool.tile([2 * cin, 3, cout], bf16)
            nc.sync.dma_start(out=wk0a[cin : 2 * cin, :, :], in_=w_body_t2.ap()[:, :, :])
            wk0b = cpool.tile([4 * cin, 3, cout], bf16)
            nc.sync.dma_start(out=wk0b[3 * cin : 4 * cin, :, :], in_=w_body_t2.ap()[:, :, :])
            # Residual + BN vectors, replicated on both partition halves.
            wr = cpool.tile([4 * cin, cout], bf16)
            nc.sync.dma_start(out=wr[0 : 2 * cin, :], in_=w_res_t.ap()[:, :])
            nc.sync.dma_start(out=wr[2 * cin : 4 * cin, :], in_=w_res_t.ap()[:, :])
            sc = cpool.tile([2 * cout, 1], f32)
            nc.sync.dma_start(out=sc[0:cout, :], in_=bn_scale.ap()[:, :])
            nc.sync.dma_start(out=sc[cout : 2 * cout, :], in_=bn_scale.ap()[:, :])
            bi = cpool.tile([2 * cout, 1], f32)
            nc.sync.dma_start(out=bi[0:cout, :], in_=bn_bias.ap()[:, :])
            nc.sync.dma_start(out=bi[cout : 2 * cout, :], in_=bn_bias.ap()[:, :])

            with (
                tc.tile_pool(name="vpool", bufs=4) as vpool,
                tc.tile_pool(name="fpool", bufs=3) as fpool,
                tc.tile_pool(name="spool", bufs=1) as spool,
                tc.tile_pool(name="opool", bufs=4) as opool,
                tc.tile_pool(name="pspool", bufs=2, space="PSUM") as pspool,
            ):
                # S buffers are managed manually (not pool-cycled) so their
                # zero-pad columns u' in {0,1} can be initialized exactly
                # once; sign/dup writes never touch them afterwards.
                n_sbufs = 6
                s_bufs = []
                for si in range(n_sbufs):
                    sb = spool.tile([128, nslots, w + 2], bf16, name=f"sbuf{si}")
                    nc.vector.memset(sb[:, :, 0:2], 0.0)
                    s_bufs.append(sb)

                reps_ctx = (
                    tc.For_i(0, loop_reps, 1) if loop_reps > 1 else nullcontext()
                )
                G = b_core * n_chunks
                with reps_ctx:
                  for pair in range(n_pairs):
                    v = vpool.tile([128, nslots, w], bf16)
                    o = opool.tile([128, chunk_rows, wo], f32)
                    halves = [h2 for h2 in range(2) if 2 * pair + h2 < G]
                    for q in halves:
                        g = 2 * pair + q
                        s = s_bufs[g % n_sbufs]
                        b, c = divmod(g, n_chunks)
                        pv = 64 * q  # V/S/output half base
                        y0 = c * chunk_rows
                        ps = pspool.tile([128, T, 512], f32)

                        # leading row slot holds zeros at c=0 (host-filled)
                        jlo = 1 if c == 0 else 0
                        if ablate != "no_in":
                            if in_path == "swdge":
                                # Per-chunk 64-partition cast-DMA (f32->bf16
                                # in the DMA datapath).
                                nc.gpsimd.dma_start(
                                    out=v[pv : pv + 64, :, :],
                                    in_=zz.ap()[pair, pv : pv + 64, :, :],
                                )
                            else:
                                # HWDGE f32 load (alternating rings) + DVE
                                # down-convert; avoids the SWDGE cast path.
                                f = fpool.tile([128, nslots, w], f32)
                                in_eng = nc.sync if q == 0 else nc.scalar
                                in_eng.dma_start(
                                    out=f[pv : pv + 64, :, :],
                                    in_=zz.ap()[pair, pv : pv + 64, :, :],
                                )
                                nc.vector.tensor_copy(
                                    v[pv : pv + 64, :, :], f[pv : pv + 64, :, :]
                                )
                            # sign bits: s = (v & 0x8000) | 0x3f80 (+-1 bf16)
                            nc.vector.tensor_scalar(
                                s.bitcast(u16)[pv : pv + 64, :, 2 : w + 2],
                                v.bitcast(u16)[pv : pv + 64, :, :],
                                0x8000,
                                0x3F80,
                                mybir.AluOpType.bitwise_and,
                                mybir.AluOpType.bitwise_or,
                            )
                        if ablate == "io_only":
                            continue

                        # Matmuls. Tap kx reads u' = 2X+kx+1 (kx=0 at X=0
                        # hits the zero pad). ky1/ky2 taps: K=64 over this
                        # half's sign partitions; the ky=0 tap (input row
                        # 2Y-1) reads the par=1 quarter one row slot back as
                        # K=32 -- no duplication needed. Weight-outer order
                        # so each LDWEIGHTS serves T consecutive matmuls.
                        # Even chunks use PE columns 0:64, odd 64:128.
                        pc = pv  # PSUM column group base = 64*q
                        w12 = wba if q == 0 else wbb
                        wk0 = wk0a if q == 0 else wk0b
                        pk = pv + cin  # par=1 quarter (ky0 tap source)
                        for kx in range(3):
                            cols = slice(kx + 1, kx + 2 * wo, 2)
                            for t in range(T):
                                j0 = 1 + 4 * t
                                nc.tensor.matmul(
                                    ps[pc : pc + 64, t, 0 : 4 * wo],
                                    w12[pv : pv + 2 * cin, kx, :],
                                    s[pv : pv + 2 * cin, j0 : j0 + 4, cols],
                                    start=(kx == 0), stop=False,
                                    tile_position=(pv, pc),
                                )
                        for kx in range(3):
                            cols = slice(kx + 1, kx + 2 * wo, 2)
                            for t in range(T):
                                j0 = 1 + 4 * t
                                if c == 0 and t == 0:
                                    # Y=0: ky=0 tap reads input row -1 (zero
                                    # pad) -> drop that output row.
                                    nc.tensor.matmul(
                                        ps[pc : pc + 64, t, wo : 4 * wo],
                                        wk0[pk : pk + cin, kx, :],
                                        s[pk : pk + cin, j0 : j0 + 3, cols],
                                        start=False, stop=False,
                                        tile_position=(pk, pc),
                                    )
                                else:
                                    nc.tensor.matmul(
                                        ps[pc : pc + 64, t, 0 : 4 * wo],
                                        wk0[pk : pk + cin, kx, :],
                                        s[pk : pk + cin, j0 - 1 : j0 + 3, cols],
                                        start=False, stop=False,
                                        tile_position=(pk, pc),
                                    )
                        for dx in range(2):
                            for t in range(T):
                                j0 = 1 + 4 * t
                                nc.tensor.matmul(
                                    ps[pc : pc + 64, t, 0 : 4 * wo],
                                    wr[2 * cin * q : 2 * cin * (q + 1), :],
                                    v[pv : pv + 64, j0 : j0 + 4, dx : dx + w - 1 : 2],
                                    start=False,
                                    stop=(dx == 1),
                                    tile_position=(pv, pc),
                                )

                        # BN + evacuate: out = psum*inv1 + (shift1+shift2)
                        nc.scalar.activation(
                            o[pv : pv + 64].rearrange("p (t j) x -> p t (j x)", t=T),
                            ps[pc : pc + 64, :, 0 : 4 * wo],
                            mybir.ActivationFunctionType.Identity,
                            bias=bi[cout * q : cout * (q + 1), :],
                            scale=sc[cout * q : cout * (q + 1), :],
                        )
                        # Per-chunk output store; alternate HWDGE rings.
                        out_eng = nc.sync if q == 0 else nc.scalar
                        out_eng.dma_start(
                            out=out.ap()[b, :, y0 : y0 + chunk_rows, :],
                            in_=o[pv : pv + 64, :, :],
                        )
    nc.compile()
    return nc


def prep_weights(w_body, w_ds, bn1_gamma, bn1_beta, bn1_mean, bn1_var,
                 bn2_gamma, bn2_beta, bn2_mean, bn2_var):
    """Host-side parameter folding (all small tensors)."""
    cout, cin = w_body.shape[0], w_body.shape[1]
    inv1 = (bn1_gamma / np.sqrt(bn1_var + EPS)).astype(np.float32)
    inv2 = (bn2_gamma / np.sqrt(bn2_var + EPS)).astype(np.float32)
    shift1 = (bn1_beta - bn1_mean * inv1).astype(np.float32)
    shift2 = (bn2_beta - bn2_mean * inv2).astype(np.float32)

    wb_sign = np.where(w_body >= 0, 1.0, -1.0).astype(np.float32)  # [co,ci,3,3]

    def body_lhst(ky_order):
        wt = np.empty((len(ky_order) * cin, 3, cout), dtype=np.float32)
        for m, ky in enumerate(ky_order):
            # [co, ci, kx] -> [ci, kx, co]
            wt[m * cin : (m + 1) * cin] = wb_sign[:, :, ky, :].transpose(1, 2, 0)
        return wt.astype(ml_dtypes.bfloat16)

    # Residual weights with BN2 folded and divided by BN1 scale (the final
    # activation multiplies everything by inv1).
    wr = w_ds[:, :, 0, 0] * (inv2 / (4.0 * inv1))[:, None]  # [co, ci]
    w_res_t = np.tile(wr.T, (2, 1)).astype(np.float32)  # [(par ci), co]

    return dict(
        w_body_t=body_lhst((1, 2)),   # direct taps (K=64 matmuls)
        w_body_t2=body_lhst((0,)),    # ky=0 tap (K=32 matmuls, row slot -1)
        w_res_t=w_res_t.astype(ml_dtypes.bfloat16),
        bn_scale=inv1.reshape(cout, 1),
        bn_bias=(shift1 + shift2).reshape(cout, 1),
    )


def make_zz(x, cin=CIN, h=H, w=W, chunk_rows=16):
    """Host layout prep: per-chunk-pair DMA payloads.

    x: [b, ci, r, u] f32. Returns zz[pair, p, slot, u] where partition
    p = 64*(chunk parity) + par*ci-major, slot j holds input row
    2*(16*c - 1 + j) + par; the leading slot of chunk 0 is zero padding.
    """
    b_core = x.shape[0]
    hh = h // 2
    n_chunks = hh // chunk_rows
    ns = chunk_rows + 1
    xv = x.reshape(b_core, cin, hh, 2, w).transpose(0, 3, 1, 2, 4).reshape(
        b_core, 2 * cin, hh, w)
    G = b_core * n_chunks
    zz = np.zeros(((G + 1) // 2, 128, ns, w), np.float32)
    for g in range(G):
        b, c = divmod(g, n_chunks)
        q, y0 = g % 2, c * chunk_rows
        jlo = 1 if c == 0 else 0
        zz[g // 2, 64 * q : 64 * q + 64, jlo:ns] = xv[
            b, :, y0 - 1 + jlo : y0 + chunk_rows, :]
    return zz


def kernel(x, w_body, bn1_gamma, bn1_beta, bn1_mean, bn1_var,
           w_ds, bn2_gamma, bn2_beta, bn2_mean, bn2_var):
    from concourse.bass_utils import run_bass_kernel_spmd

    x = np.asarray(x, dtype=np.float32)
    params = prep_weights(
        np.asarray(w_body, np.float32), np.asarray(w_ds, np.float32),
        np.asarray(bn1_gamma, np.float32), np.asarray(bn1_beta, np.float32),
        np.asarray(bn1_mean, np.float32), np.asarray(bn1_var, np.float32),
        np.asarray(bn2_gamma, np.float32), np.asarray(bn2_beta, np.float32),
        np.asarray(bn2_mean, np.float32), np.asarray(bn2_var, np.float32),
    )

    nc = build_nc()
    in_maps = [
        {"zz": make_zz(x[k * B_CORE : (k + 1) * B_CORE]), **params}
        for k in range(N_CORES)
    ]
    res = run_bass_kernel_spmd(nc, in_maps, core_ids=list(range(N_CORES)))
    return np.concatenate([r["out"] for r in res.results], axis=0)
